# revision 2
# baseline (speedup 1.0000x reference)
"""Trainium2 Bass kernel for batched differentiable mean-variance optimization.

Problem: for each of 256 samples, solve
    min 0.5 y^T Sigma y  s.t.  mu^T y = 1, y >= 0
via 150 unrolled projected-gradient iterations (step = 1/lambda_max via power
iteration), then normalize to portfolio weights.  Pure data parallel: 32
samples per core on 8 cores.

v2 design (vs v1 fp32r):
- Sigma resident in SBUF as bf16 for all 32 samples (16 MB).  bf16 matmul
  streams at 1 cyc/row and allows 4 concurrent samples in distinct 32-column
  groups of the PE array (tile_position) -> ~4x matvec throughput.
- Matvec psum output rows {32*db} for 2 quarter-sets share a [128,2,512] psum
  pair tile; one ScalarE copy [97,2,512] stages it, one DMA scatters 8
  samples into the A4 projection layout (partition pi = 4*slot + quarter).
- Projection (semismooth Newton on the simplex-like constraint) runs on DVE
  in per-half [64,128] A4 tiles; per-sample sums via a block-diag G matmul
  (sum over the 4 quarter-partitions of a sample + broadcast back).
- The two halves (16 samples each) are software-pipelined: half h's matvec
  streams on PE while half 1-h runs Newton on DVE; the small G matmuls are
  emitted interleaved between matvec groups so PE never waits long.
- Slot permutation: DRAM sample b = 8P + 4s + db lives at slot j = 8P+2db+s
  (pi = 4j + q).  Host-visible DMAs (mu in, w out) undo it with rearranged
  DRAM access patterns; sigma load permutes in the Python loop.
"""

import os
import numpy as np
from contextlib import ExitStack

N = 512
NCORES = 8
SPC = 32           # samples per core
POWER_ITERS = 10
PGD_ITERS = 150
NEWTON_K = 4

_PROGRAM_CACHE = {}


def _slot_to_dram(j):
    """x_B/sigma slot j -> DRAM sample row (per core)."""
    P, r = divmod(j, 8)
    db, s = divmod(r, 2)
    return 8 * P + 4 * s + db


def _build_program(power_iters=POWER_ITERS, pgd_iters=PGD_ITERS,
                   newton_k=NEWTON_K):
    import concourse.bacc as bacc
    import concourse.tile as tile
    from concourse import mybir

    Alu = mybir.AluOpType
    F32 = mybir.dt.float32
    BF16 = mybir.dt.bfloat16

    nc = bacc.Bacc(
        "TRN2",
        target_bir_lowering=False,
        debug=False,
        enable_asserts=False,
        num_devices=NCORES,
    )

    mu_dram = nc.dram_tensor("mu_in", [SPC, N], F32, kind="ExternalInput").ap()
    sig_dram = nc.dram_tensor("sigma_in", [SPC, N, N], BF16,
                              kind="ExternalInput").ap()
    g64_dram = nc.dram_tensor("g64_in", [64, 64], F32, kind="ExternalInput").ap()
    id64_dram = nc.dram_tensor("id64_in", [64, 64], F32, kind="ExternalInput").ap()
    w_dram = nc.dram_tensor("w_out", [SPC, N], F32, kind="ExternalOutput").ap()

    # DRAM sample b = 8P + 4s + d lives at A4 partitions 32P' + 8d + 4s + q
    # (q = 0..3 contiguous).  Host-visible transfers go one DMA per sample:
    # DRAM [1, 512] <-> A4 [4 contiguous partitions, 128].
    def a4_sample_range(Pp, s, dd):
        base = 32 * Pp + 8 * dd + 4 * s
        return base, base + 4

    def half_samples(h):
        for Pp in range(2):
            for s in range(2):
                for dd in range(4):
                    b = 8 * (2 * h + Pp) + 4 * s + dd
                    yield Pp, s, dd, b

    with tile.TileContext(nc) as tc, ExitStack() as ctx:
        const_pool = ctx.enter_context(tc.tile_pool(name="const", bufs=1))
        sig_pool = ctx.enter_context(tc.tile_pool(name="sig", bufs=1))
        state_pool = ctx.enter_context(tc.tile_pool(name="state", bufs=1))
        adma_pool = ctx.enter_context(tc.tile_pool(name="adma", bufs=2))
        mv_pool = ctx.enter_context(tc.tile_pool(name="mv", bufs=1, space="PSUM"))
        tr_pool = ctx.enter_context(tc.tile_pool(name="tr", bufs=1, space="PSUM"))
        nw_pool = ctx.enter_context(tc.tile_pool(name="nw", bufs=1, space="PSUM"))

        g64_sb = const_pool.tile([64, 64], F32)
        nc.sync.dma_start(out=g64_sb, in_=g64_dram)
        id64_sb = const_pool.tile([64, 64], F32)
        nc.sync.dma_start(out=id64_sb, in_=id64_dram)

        # Sigma resident: [part p, slot j, chunk c, elem e] = Sigma[b(j)][128c+p, e]
        sig_sb = sig_pool.tile([128, SPC, 4, N], BF16)
        for j in range(SPC):
            nc.sync.dma_start(
                out=sig_sb[:, j],
                in_=sig_dram[_slot_to_dram(j)].rearrange("(c p) e -> p c e", p=128),
            )

        # B layout iterate: partition = element within quarter, free (slot, q).
        x_B = state_pool.tile([128, SPC, 4], BF16, tag="xB")

        # Per-half A4 state ([64, 128]: partition 4*j_rel + q, free elem).
        # Free dim padded to 132 so DMA AP balancing can never merge the
        # per-partition row with the partition stride into runs that would
        # cross partitions (that merge is physically wrong on SBUF).
        H = {}
        for h in (0, 1):
            d = {}
            for nm in ("mu", "imu", "msq", "g", "ys", "u", "r", "muv", "t",
                       "yfin", "va4", "wa4"):
                d[nm] = state_pool.tile([64, 128], F32, tag=f"h{h}_{nm}",
                                        name=f"h{h}_{nm}",
                                        padded_shape=[64, 132])
            d["prod"] = state_pool.tile([64, 2, 128], F32, tag=f"h{h}_prod",
                                        name=f"h{h}_prod")
            for nm in ("ab", "nd"):
                d[nm] = state_pool.tile([64, 2], F32, tag=f"h{h}_{nm}",
                                        name=f"h{h}_{nm}")
            for nm in ("neglam", "lam", "rb", "bmax", "negstep", "invnegstep",
                       "cnt", "mvd", "omv", "sp", "ok", "sc", "off", "s2"):
                d[nm] = state_pool.tile([64, 1], F32, tag=f"h{h}_{nm}",
                                        name=f"h{h}_{nm}")
            H[h] = d

        if os.environ.get("KM_SIM_SAFE"):
            # The interpreter's init tracking can't merge scattered DMA
            # writes; pre-fill scatter targets (sim correctness aid only).
            for h in (0, 1):
                for nm in ("mu", "g", "va4", "wa4"):
                    nc.vector.memset(H[h][nm], 0.0)

        for h in (0, 1):
            d = H[h]
            for Pp, s, dd, b in half_samples(h):
                p0, p1 = a4_sample_range(Pp, s, dd)
                nc.sync.dma_start(out=d["mu"][p0:p1, :],
                                  in_=mu_dram[b: b + 1, :])
            nc.vector.reciprocal(d["imu"], d["mu"])
            nc.vector.tensor_mul(d["msq"], d["mu"], d["mu"])

        # ---------- matvec ----------
        def matvec_stages(h, dst):
            """Return a list of closures; running all of them emits the matvec
            of half h's 16 slots into A4 tile `dst` ([64,128]).  Split into
            stages so Newton gmms of the other half can interleave."""
            stages = []
            for Pp in (0, 1):          # quarter-pair within the half
                P = 2 * h + Pp

                def emit_mm(P=P, Pp=Pp):
                    ps = mv_pool.tile([128, 2, 512], F32, tag=f"mv{P % 2}",
                                      name=f"mv{P % 2}")
                    if os.environ.get("KM_SIM_SAFE"):
                        nc.vector.memset(ps, 0.0)
                    for s in range(2):
                        for p in range(4):
                            for db in range(4):
                                j = 8 * P + 2 * db + s
                                nc.tensor.matmul(
                                    ps[32 * db: 32 * db + 1, s, :],
                                    x_B[:, j, p: p + 1],
                                    sig_sb[:, j, p, :],
                                    start=(p == 0),
                                    stop=(p == 3),
                                    tile_position=(0, 32 * db),
                                )
                    stage = adma_pool.tile([128, 2, 512], F32, tag=f"st{P % 2}",
                                           name=f"st{P % 2}", bufs=2)
                    nc.scalar.copy(stage[0:97], ps[0:97])
                    # src [4, 1024] / dst [32, 128]: the balancer refines to
                    # the common (d, sq, f) 3-dim form.
                    nc.sync.dma_start(
                        out=dst[32 * Pp: 32 * Pp + 32, :],
                        in_=stage[0:97:32].rearrange("d s f -> d (s f)"),
                    )
                stages.append(emit_mm)
            return stages

        def transpose_to_xB(h, src_a4):
            trp = tr_pool.tile([128, 64], F32, tag=f"tr{h}", name=f"tr{h}")
            nc.tensor.transpose(trp, src_a4, id64_sb)
            nc.vector.tensor_copy(
                x_B[:, 16 * h: 16 * h + 16, :],
                trp.rearrange("p (j q) -> p j q", q=4),
            )

        def gmm(h, rhs, out_ps, n):
            nc.tensor.matmul(out_ps[:, 0:n], g64_sb, rhs[:, 0:n],
                             start=True, stop=True)

        # ---------- Newton projection ----------
        def newton_stages(h, r_ap, muv_ap):
            """Closure list for newton_k iterations of the lam solve for half
            h.  Each gmm is its own stage so it can interleave with matvec
            matmuls on the PE queue."""
            d = H[h]
            stages = []
            for _ in range(newton_k):
                def dve_part(d=d, r_ap=r_ap, muv_ap=muv_ap):
                    nc.vector.scalar_tensor_tensor(
                        out=d["prod"][:, 0, :], in0=r_ap,
                        scalar=d["neglam"][:, 0:1], in1=muv_ap,
                        op0=Alu.is_gt, op1=Alu.mult, accum_out=d["ab"][:, 0:1],
                    )
                    nc.vector.scalar_tensor_tensor(
                        out=d["prod"][:, 1, :], in0=r_ap,
                        scalar=d["neglam"][:, 0:1], in1=d["msq"],
                        op0=Alu.is_gt, op1=Alu.mult, accum_out=d["ab"][:, 1:2],
                    )

                def pe_part(d=d, h=h):
                    abp = nw_pool.tile([64, 2], F32, tag=f"nw{h}", name=f"nw{h}")
                    gmm(h, d["ab"], abp, 2)
                    nc.vector.tensor_scalar(
                        out=d["bmax"], in0=abp[:, 1:2], scalar1=1e-30,
                        scalar2=None, op0=Alu.max,
                    )
                    nc.vector.reciprocal(d["rb"], d["bmax"])
                    nc.vector.scalar_tensor_tensor(
                        out=d["neglam"], in0=abp[:, 0:1], scalar=-1.0,
                        in1=d["rb"], op0=Alu.add, op1=Alu.mult,
                    )
                stages.append(dve_part)
                stages.append(pe_part)
            return stages

        def pgd_update_stages(h, k):
            """Stages for half h's iteration-k projection: pre-ops, newton,
            post-ops (+transpose back to x_B, or y_fin on the last iter)."""
            d = H[h]
            stages = []

            def pre(d=d):
                nc.vector.scalar_tensor_tensor(
                    out=d["u"], in0=d["ys"], scalar=d["invnegstep"][:, 0:1],
                    in1=d["g"], op0=Alu.mult, op1=Alu.add,
                )
                nc.vector.tensor_mul(d["r"], d["u"], d["imu"])
                nc.vector.tensor_mul(d["muv"], d["u"], d["mu"])
            stages.append(pre)
            stages.extend(newton_stages(h, H[h]["r"], H[h]["muv"]))

            def post(d=d, h=h, k=k):
                # t = lam*mu + u  (lam = -neglam)
                nc.vector.tensor_scalar(
                    out=d["lam"], in0=d["neglam"], scalar1=-1.0, scalar2=None,
                    op0=Alu.mult,
                )
                nc.vector.scalar_tensor_tensor(
                    out=d["t"], in0=d["mu"], scalar=d["lam"][:, 0:1],
                    in1=d["u"], op0=Alu.mult, op1=Alu.add,
                )
                if k < pgd_iters - 1:
                    nc.vector.tensor_scalar(
                        out=d["ys"], in0=d["t"], scalar1=0.0,
                        scalar2=d["negstep"][:, 0:1], op0=Alu.max, op1=Alu.mult,
                    )
                    transpose_to_xB(h, d["ys"])
                else:
                    nc.vector.tensor_scalar(
                        out=d["yfin"], in0=d["t"], scalar1=0.0, scalar2=None,
                        op0=Alu.max,
                    )
            stages.append(post)
            return stages

        def run_interleaved(mv_st, up_st):
            """Interleave matvec stages (PE bulk) with update stages of the
            other half, so Newton's small gmms slot between matmul groups."""
            up_i = 0
            chunks = len(mv_st)
            per = (len(up_st) + chunks - 1) // chunks if chunks else 0
            for i in range(chunks):
                mv_st[i]()
                for _ in range(per):
                    if up_i < len(up_st):
                        up_st[up_i]()
                        up_i += 1
            while up_i < len(up_st):
                up_st[up_i]()
                up_i += 1

        # ---------- power phase ----------
        nc.vector.memset(x_B, 1.0)
        for kk in range(power_iters):
            # Emit both halves' matvecs before the transposes: the transpose
            # of half 0 then lands after half 1's matmul stream on the PE
            # queue, by which time its repack DMA has long finished.
            for h in (0, 1):
                for st in matvec_stages(h, H[h]["va4"]):
                    st()
            for h in (0, 1):
                transpose_to_xB(h, H[h]["va4"])
        for h in (0, 1):
            for st in matvec_stages(h, H[h]["wa4"]):
                st()
        for h in (0, 1):
            d = H[h]
            nc.vector.scalar_tensor_tensor(
                out=d["prod"][:, 0, :], in0=d["va4"], scalar=0.0, in1=d["wa4"],
                op0=Alu.add, op1=Alu.mult, accum_out=d["nd"][:, 0:1],
            )
            nc.vector.scalar_tensor_tensor(
                out=d["prod"][:, 1, :], in0=d["va4"], scalar=0.0, in1=d["va4"],
                op0=Alu.add, op1=Alu.mult, accum_out=d["nd"][:, 1:2],
            )
            nwp = nw_pool.tile([64, 2], F32, tag=f"nw{h}", name=f"nw{h}")
            gmm(h, d["nd"], nwp, 2)
            nc.vector.reciprocal(d["rb"], nwp[:, 0:1])           # 1/(v.w)
            nc.vector.scalar_tensor_tensor(
                out=d["negstep"], in0=nwp[:, 1:2], scalar=-1.0, in1=d["rb"],
                op0=Alu.mult, op1=Alu.mult,
            )                                                    # -1/lmax
            nc.vector.reciprocal(d["bmax"], nwp[:, 1:2])         # 1/(v.v)
            nc.vector.scalar_tensor_tensor(
                out=d["invnegstep"], in0=nwp[:, 0:1], scalar=-1.0,
                in1=d["bmax"], op0=Alu.mult, op1=Alu.mult,
            )                                                    # -lmax

        # ---------- y0 = project(ones) ----------
        for h in (0, 1):
            d = H[h]
            nc.vector.memset(d["neglam"], -1e30)
            for st in newton_stages(h, d["imu"], d["mu"]):
                st()
            nc.vector.tensor_scalar(
                out=d["lam"], in0=d["neglam"], scalar1=-1.0, scalar2=None,
                op0=Alu.mult,
            )
            nc.vector.tensor_scalar(
                out=d["t"], in0=d["mu"], scalar1=d["lam"][:, 0:1], scalar2=1.0,
                op0=Alu.mult, op1=Alu.add,
            )
            nc.vector.tensor_scalar(
                out=d["ys"], in0=d["t"], scalar1=0.0,
                scalar2=d["negstep"][:, 0:1], op0=Alu.max, op1=Alu.mult,
            )
            transpose_to_xB(h, d["ys"])

        # ---------- PGD (software-pipelined halves) ----------
        # Phase (k, h): matvec_h(k) interleaved with update_{1-h} of its
        # pending iteration.
        pending = {0: None, 1: None}   # half -> iteration index awaiting update
        for k in range(pgd_iters):
            for h in (0, 1):
                other = 1 - h
                up = (pgd_update_stages(other, pending[other])
                      if pending[other] is not None else [])
                run_interleaved(matvec_stages(h, H[h]["g"]), up)
                pending[other] = None
                pending[h] = k
        for h in (0, 1):
            if pending[h] is not None:
                for st in pgd_update_stages(h, pending[h]):
                    st()
                pending[h] = None

        # ---------- postprocess ----------
        for h in (0, 1):
            d = H[h]
            nc.vector.tensor_scalar(
                out=d["prod"][:, 0, :], in0=d["mu"], scalar1=1e-6, scalar2=None,
                op0=Alu.is_gt, op1=Alu.add, accum_out=d["cnt"],
            )
            cntp = nw_pool.tile([64, 2], F32, tag=f"nw{h}", name=f"nw{h}")
            gmm(h, d["cnt"], cntp, 1)
            nc.vector.tensor_scalar(
                out=d["mvd"], in0=cntp[:, 0:1], scalar1=0.5, scalar2=None,
                op0=Alu.is_gt,
            )
            nc.vector.tensor_scalar(
                out=d["omv"], in0=d["mvd"], scalar1=-1.0, scalar2=1.0,
                op0=Alu.mult, op1=Alu.add,
            )
            y2 = d["t"]   # reuse
            nc.vector.tensor_scalar(
                out=y2, in0=d["yfin"], scalar1=d["mvd"][:, 0:1],
                scalar2=d["omv"][:, 0:1], op0=Alu.mult, op1=Alu.add,
            )
            nc.vector.tensor_scalar(
                out=d["prod"][:, 0, :], in0=y2, scalar1=1.0, scalar2=None,
                op0=Alu.mult, op1=Alu.add, accum_out=d["sp"],
            )
            spp = nw_pool.tile([64, 2], F32, tag=f"nw{h}", name=f"nw{h}")
            gmm(h, d["sp"], spp, 1)
            nc.vector.tensor_scalar(
                out=d["ok"], in0=spp[:, 0:1], scalar1=1e-6, scalar2=None,
                op0=Alu.is_gt,
            )
            nc.vector.tensor_scalar(
                out=d["bmax"], in0=spp[:, 0:1], scalar1=1e-30, scalar2=None,
                op0=Alu.max,
            )
            nc.vector.reciprocal(d["rb"], d["bmax"])
            nc.vector.tensor_mul(d["sc"], d["rb"], d["ok"])
            nc.vector.tensor_scalar(
                out=d["off"], in0=d["ok"], scalar1=-1.0 / N, scalar2=1.0 / N,
                op0=Alu.mult, op1=Alu.add,
            )
            w1 = d["u"]   # reuse
            nc.vector.tensor_scalar(
                out=w1, in0=y2, scalar1=d["sc"][:, 0:1],
                scalar2=d["off"][:, 0:1], op0=Alu.mult, op1=Alu.add,
            )
            nc.vector.tensor_scalar(
                out=d["prod"][:, 0, :], in0=w1, scalar1=1.0, scalar2=None,
                op0=Alu.mult, op1=Alu.add, accum_out=d["s2"],
            )
            s2p = nw_pool.tile([64, 2], F32, tag=f"nw{h}", name=f"nw{h}")
            gmm(h, d["s2"], s2p, 1)
            nc.vector.reciprocal(d["rb"], s2p[:, 0:1])
            wf = d["r"]   # reuse
            nc.vector.tensor_scalar(
                out=wf, in0=w1, scalar1=d["rb"][:, 0:1], scalar2=None,
                op0=Alu.mult,
            )
            for Pp, s, dd, b in half_samples(h):
                p0, p1 = a4_sample_range(Pp, s, dd)
                nc.sync.dma_start(out=w_dram[b: b + 1, :],
                                  in_=wf[p0:p1, :])

    nc.compile()
    return nc


def _get_program():
    if "nc" not in _PROGRAM_CACHE:
        _PROGRAM_CACHE["nc"] = _build_program()
    return _PROGRAM_CACHE["nc"]


def _host_inputs(mu, sig):
    import ml_dtypes
    sig_bf = sig.astype(ml_dtypes.bfloat16)
    g64 = np.kron(np.eye(16, dtype=np.float32), np.ones((4, 4), np.float32))
    id64 = np.eye(64, dtype=np.float32)
    return sig_bf, g64, id64


def kernel(predicted_returns: np.ndarray, covariance_matrix: np.ndarray) -> np.ndarray:
    from concourse.bass_utils import run_bass_kernel_spmd

    mu = np.ascontiguousarray(predicted_returns, dtype=np.float32)
    sig = np.ascontiguousarray(covariance_matrix, dtype=np.float32)
    batch = mu.shape[0]
    assert batch == NCORES * SPC and mu.shape[1] == N

    sig_bf, g64, id64 = _host_inputs(mu, sig)

    nc = _get_program()
    in_maps = []
    for c in range(NCORES):
        sl = slice(c * SPC, (c + 1) * SPC)
        in_maps.append({
            "mu_in": mu[sl],
            "sigma_in": sig_bf[sl],
            "g64_in": g64,
            "id64_in": id64,
        })
    res = run_bass_kernel_spmd(nc, in_maps, core_ids=list(range(NCORES)))
    out = np.concatenate([r["w_out"] for r in res.results], axis=0)
    return out.astype(np.float32)


if __name__ == "__main__":
    rng = np.random.default_rng(0)
    mu = (0.05 + 0.1 * rng.random((NCORES * SPC, N))).astype(np.float32)
    A = rng.standard_normal((4, N, N)).astype(np.float32)
    sig = np.einsum("bik,bjk->bij", A, A) / N + 0.1 * np.eye(N, dtype=np.float32)
    sig = np.tile(sig, (64, 1, 1)).astype(np.float32)
    w = kernel(mu, sig)
    print(w.shape, w.sum(axis=1)[:4])


# revision 3
# speedup vs baseline: 1.8583x; 1.8583x over previous
"""Trainium2 Bass kernel for batched differentiable mean-variance optimization.

Problem: for each of 256 samples, solve
    min 0.5 y^T Sigma y  s.t.  mu^T y = 1, y >= 0
via 150 unrolled projected-gradient iterations (step = 1/lambda_max via power
iteration), then normalize to portfolio weights.  Pure data parallel: 32
samples per core on 8 cores.

v2 design (vs v1 fp32r):
- Sigma resident in SBUF as bf16 for all 32 samples (16 MB).  bf16 matmul
  streams at 1 cyc/row and allows 4 concurrent samples in distinct 32-column
  groups of the PE array (tile_position) -> ~4x matvec throughput.
- Matvec psum output rows {32*db} for 2 quarter-sets share a [128,2,512] psum
  pair tile; one ScalarE copy [97,2,512] stages it, one DMA scatters 8
  samples into the A4 projection layout (partition pi = 4*slot + quarter).
- Projection (semismooth Newton on the simplex-like constraint) runs on DVE
  in per-half [64,128] A4 tiles; per-sample sums via a block-diag G matmul
  (sum over the 4 quarter-partitions of a sample + broadcast back).
- The two halves (16 samples each) are software-pipelined: half h's matvec
  streams on PE while half 1-h runs Newton on DVE; the small G matmuls are
  emitted interleaved between matvec groups so PE never waits long.
- Slot permutation: DRAM sample b = 8P + 4s + db lives at slot j = 8P+2db+s
  (pi = 4j + q).  Host-visible DMAs (mu in, w out) undo it with rearranged
  DRAM access patterns; sigma load permutes in the Python loop.
"""

import os
import numpy as np
from contextlib import ExitStack

N = 512
NCORES = 8
SPC = 32           # samples per core
POWER_ITERS = 8
PGD_ITERS = 150
NEWTON_K = 3

_PROGRAM_CACHE = {}


def _slot_to_dram(j):
    """x_B/sigma slot j -> DRAM sample row (per core)."""
    P, r = divmod(j, 8)
    db, s = divmod(r, 2)
    return 8 * P + 4 * s + db


def _build_program(power_iters=POWER_ITERS, pgd_iters=PGD_ITERS,
                   newton_k=NEWTON_K):
    import concourse.bacc as bacc
    import concourse.tile as tile
    from concourse import mybir

    Alu = mybir.AluOpType
    F32 = mybir.dt.float32
    BF16 = mybir.dt.bfloat16

    nc = bacc.Bacc(
        "TRN2",
        target_bir_lowering=False,
        debug=False,
        enable_asserts=False,
        num_devices=NCORES,
    )

    mu_dram = nc.dram_tensor("mu_in", [SPC, N], F32, kind="ExternalInput").ap()
    sig_dram = nc.dram_tensor("sigma_in", [SPC, N, N], BF16,
                              kind="ExternalInput").ap()
    g64_dram = nc.dram_tensor("g64_in", [64, 64], F32, kind="ExternalInput").ap()
    id64_dram = nc.dram_tensor("id64_in", [64, 64], F32, kind="ExternalInput").ap()
    w_dram = nc.dram_tensor("w_out", [SPC, N], F32, kind="ExternalOutput").ap()

    # DRAM sample b = 8P + 4s + d lives at A4 partitions 32P' + 8d + 4s + q
    # (q = 0..3 contiguous).  Host-visible transfers go one DMA per sample:
    # DRAM [1, 512] <-> A4 [4 contiguous partitions, 128].
    def a4_sample_range(Pp, s, dd):
        base = 32 * Pp + 8 * dd + 4 * s
        return base, base + 4

    def half_samples(h):
        for Pp in range(2):
            for s in range(2):
                for dd in range(4):
                    b = 8 * (2 * h + Pp) + 4 * s + dd
                    yield Pp, s, dd, b

    with tile.TileContext(nc) as tc, ExitStack() as ctx:
        const_pool = ctx.enter_context(tc.tile_pool(name="const", bufs=1))
        sig_pool = ctx.enter_context(tc.tile_pool(name="sig", bufs=1))
        state_pool = ctx.enter_context(tc.tile_pool(name="state", bufs=1))
        adma_pool = ctx.enter_context(tc.tile_pool(name="adma", bufs=2))
        mv_pool = ctx.enter_context(tc.tile_pool(name="mv", bufs=1, space="PSUM"))
        tr_pool = ctx.enter_context(tc.tile_pool(name="tr", bufs=1, space="PSUM"))
        nw_pool = ctx.enter_context(tc.tile_pool(name="nw", bufs=1, space="PSUM"))

        g64_sb = const_pool.tile([64, 64], F32)
        nc.sync.dma_start(out=g64_sb, in_=g64_dram)
        id64_sb = const_pool.tile([64, 64], F32)
        nc.sync.dma_start(out=id64_sb, in_=id64_dram)

        # Sigma resident: [part p, slot j, chunk c, elem e] = Sigma[b(j)][128c+p, e]
        sig_sb = sig_pool.tile([128, SPC, 4, N], BF16)
        for j in range(SPC):
            nc.sync.dma_start(
                out=sig_sb[:, j],
                in_=sig_dram[_slot_to_dram(j)].rearrange("(c p) e -> p c e", p=128),
            )

        # B layout iterate: partition = element within quarter, free (slot, q).
        x_B = state_pool.tile([128, SPC, 4], BF16, tag="xB")

        # Per-half A4 state ([64, 128]: partition 4*j_rel + q, free elem).
        # Free dim padded to 132 so DMA AP balancing can never merge the
        # per-partition row with the partition stride into runs that would
        # cross partitions (that merge is physically wrong on SBUF).
        H = {}
        for h in (0, 1):
            d = {}
            for nm in ("mu", "imu", "msq", "g", "ys", "u", "r", "muv", "t",
                       "yfin", "va4", "wa4"):
                d[nm] = state_pool.tile([64, 128], F32, tag=f"h{h}_{nm}",
                                        name=f"h{h}_{nm}",
                                        padded_shape=[64, 132])
            d["prod"] = state_pool.tile([64, 2, 128], F32, tag=f"h{h}_prod",
                                        name=f"h{h}_prod")
            for nm in ("ab", "nd"):
                d[nm] = state_pool.tile([64, 2], F32, tag=f"h{h}_{nm}",
                                        name=f"h{h}_{nm}")
            for nm in ("neglam", "lam", "rb", "bmax", "negstep", "invnegstep",
                       "cnt", "mvd", "omv", "sp", "ok", "sc", "off", "s2"):
                d[nm] = state_pool.tile([64, 1], F32, tag=f"h{h}_{nm}",
                                        name=f"h{h}_{nm}")
            H[h] = d

        if os.environ.get("KM_SIM_SAFE"):
            # The interpreter's init tracking can't merge scattered DMA
            # writes; pre-fill scatter targets (sim correctness aid only).
            for h in (0, 1):
                for nm in ("mu", "g", "va4", "wa4"):
                    nc.vector.memset(H[h][nm], 0.0)

        for h in (0, 1):
            d = H[h]
            for Pp, s, dd, b in half_samples(h):
                p0, p1 = a4_sample_range(Pp, s, dd)
                nc.sync.dma_start(out=d["mu"][p0:p1, :],
                                  in_=mu_dram[b: b + 1, :])
            nc.vector.reciprocal(d["imu"], d["mu"])
            nc.vector.tensor_mul(d["msq"], d["mu"], d["mu"])

        # ---------- matvec ----------
        def matvec_stages(h, dst):
            """Return a list of closures; running all of them emits the matvec
            of half h's 16 slots into A4 tile `dst` ([64,128]).  Split into
            stages so Newton gmms of the other half can interleave."""
            stages = []
            for Pp in (0, 1):          # quarter-pair within the half
                P = 2 * h + Pp

                def emit_mm(P=P, Pp=Pp):
                    ps = mv_pool.tile([128, 2, 512], F32, tag=f"mv{P % 2}",
                                      name=f"mv{P % 2}")
                    if os.environ.get("KM_SIM_SAFE"):
                        nc.vector.memset(ps, 0.0)
                    for s in range(2):
                        for p in range(4):
                            for db in range(4):
                                j = 8 * P + 2 * db + s
                                nc.tensor.matmul(
                                    ps[32 * db: 32 * db + 1, s, :],
                                    x_B[:, j, p: p + 1],
                                    sig_sb[:, j, p, :],
                                    start=(p == 0),
                                    stop=(p == 3),
                                    tile_position=(0, 32 * db),
                                )
                    stage = adma_pool.tile([128, 2, 512], F32, tag=f"st{P % 2}",
                                           name=f"st{P % 2}", bufs=2)
                    nc.scalar.copy(stage[0:97], ps[0:97])
                    # src [4, 1024] / dst [32, 128]: the balancer refines to
                    # the common (d, sq, f) 3-dim form.
                    nc.sync.dma_start(
                        out=dst[32 * Pp: 32 * Pp + 32, :],
                        in_=stage[0:97:32].rearrange("d s f -> d (s f)"),
                    )
                stages.append(emit_mm)
            return stages

        def transpose_to_xB(h, src_a4):
            trp = tr_pool.tile([128, 64], F32, tag=f"tr{h}", name=f"tr{h}")
            nc.tensor.transpose(trp, src_a4, id64_sb)
            nc.vector.tensor_copy(
                x_B[:, 16 * h: 16 * h + 16, :],
                trp.rearrange("p (j q) -> p j q", q=4),
            )

        def gmm(h, rhs, out_ps, n):
            nc.tensor.matmul(out_ps[:, 0:n], g64_sb, rhs[:, 0:n],
                             start=True, stop=True)

        # ---------- Newton projection ----------
        def newton_stages(h, r_ap, muv_ap):
            """Closure list for newton_k iterations of the lam solve for half
            h.  Each gmm is its own stage so it can interleave with matvec
            matmuls on the PE queue."""
            d = H[h]
            stages = []
            for _ in range(newton_k):
                def dve_part(d=d, r_ap=r_ap, muv_ap=muv_ap):
                    nc.vector.scalar_tensor_tensor(
                        out=d["prod"][:, 0, :], in0=r_ap,
                        scalar=d["neglam"][:, 0:1], in1=muv_ap,
                        op0=Alu.is_gt, op1=Alu.mult, accum_out=d["ab"][:, 0:1],
                    )
                    nc.vector.scalar_tensor_tensor(
                        out=d["prod"][:, 1, :], in0=r_ap,
                        scalar=d["neglam"][:, 0:1], in1=d["msq"],
                        op0=Alu.is_gt, op1=Alu.mult, accum_out=d["ab"][:, 1:2],
                    )

                def pe_part(d=d, h=h):
                    abp = nw_pool.tile([64, 2], F32, tag=f"nw{h}", name=f"nw{h}")
                    gmm(h, d["ab"], abp, 2)
                    nc.vector.tensor_scalar(
                        out=d["bmax"], in0=abp[:, 1:2], scalar1=1e-30,
                        scalar2=None, op0=Alu.max,
                    )
                    nc.vector.reciprocal(d["rb"], d["bmax"])
                    nc.vector.scalar_tensor_tensor(
                        out=d["neglam"], in0=abp[:, 0:1], scalar=-1.0,
                        in1=d["rb"], op0=Alu.add, op1=Alu.mult,
                    )
                stages.append(dve_part)
                stages.append(pe_part)
            return stages

        def pgd_update_stages(h, k):
            """Stages for half h's iteration-k projection: pre-ops, newton,
            post-ops (+transpose back to x_B, or y_fin on the last iter)."""
            d = H[h]
            stages = []

            def pre(d=d):
                nc.vector.scalar_tensor_tensor(
                    out=d["u"], in0=d["ys"], scalar=d["invnegstep"][:, 0:1],
                    in1=d["g"], op0=Alu.mult, op1=Alu.add,
                )
                nc.vector.tensor_mul(d["r"], d["u"], d["imu"])
                nc.vector.tensor_mul(d["muv"], d["u"], d["mu"])
            stages.append(pre)
            stages.extend(newton_stages(h, H[h]["r"], H[h]["muv"]))

            def post(d=d, h=h, k=k):
                # t = lam*mu + u  (lam = -neglam)
                nc.vector.tensor_scalar(
                    out=d["lam"], in0=d["neglam"], scalar1=-1.0, scalar2=None,
                    op0=Alu.mult,
                )
                nc.vector.scalar_tensor_tensor(
                    out=d["t"], in0=d["mu"], scalar=d["lam"][:, 0:1],
                    in1=d["u"], op0=Alu.mult, op1=Alu.add,
                )
                if k < pgd_iters - 1:
                    nc.vector.tensor_scalar(
                        out=d["ys"], in0=d["t"], scalar1=0.0,
                        scalar2=d["negstep"][:, 0:1], op0=Alu.max, op1=Alu.mult,
                    )
                    transpose_to_xB(h, d["ys"])
                else:
                    nc.vector.tensor_scalar(
                        out=d["yfin"], in0=d["t"], scalar1=0.0, scalar2=None,
                        op0=Alu.max,
                    )
            stages.append(post)
            return stages

        def run_interleaved(mv_st, up_st):
            """Interleave matvec stages (PE bulk) with update stages of the
            other half, so Newton's small gmms slot between matmul groups."""
            up_i = 0
            chunks = len(mv_st)
            per = (len(up_st) + chunks - 1) // chunks if chunks else 0
            for i in range(chunks):
                mv_st[i]()
                for _ in range(per):
                    if up_i < len(up_st):
                        up_st[up_i]()
                        up_i += 1
            while up_i < len(up_st):
                up_st[up_i]()
                up_i += 1

        # ---------- power phase ----------
        nc.vector.memset(x_B, 1.0)
        for kk in range(power_iters):
            # Emit both halves' matvecs before the transposes: the transpose
            # of half 0 then lands after half 1's matmul stream on the PE
            # queue, by which time its repack DMA has long finished.
            for h in (0, 1):
                for st in matvec_stages(h, H[h]["va4"]):
                    st()
            for h in (0, 1):
                transpose_to_xB(h, H[h]["va4"])
        for h in (0, 1):
            for st in matvec_stages(h, H[h]["wa4"]):
                st()
        for h in (0, 1):
            d = H[h]
            nc.vector.scalar_tensor_tensor(
                out=d["prod"][:, 0, :], in0=d["va4"], scalar=0.0, in1=d["wa4"],
                op0=Alu.add, op1=Alu.mult, accum_out=d["nd"][:, 0:1],
            )
            nc.vector.scalar_tensor_tensor(
                out=d["prod"][:, 1, :], in0=d["va4"], scalar=0.0, in1=d["va4"],
                op0=Alu.add, op1=Alu.mult, accum_out=d["nd"][:, 1:2],
            )
            nwp = nw_pool.tile([64, 2], F32, tag=f"nw{h}", name=f"nw{h}")
            gmm(h, d["nd"], nwp, 2)
            nc.vector.reciprocal(d["rb"], nwp[:, 0:1])           # 1/(v.w)
            nc.vector.scalar_tensor_tensor(
                out=d["negstep"], in0=nwp[:, 1:2], scalar=-1.0, in1=d["rb"],
                op0=Alu.mult, op1=Alu.mult,
            )                                                    # -1/lmax
            nc.vector.reciprocal(d["bmax"], nwp[:, 1:2])         # 1/(v.v)
            nc.vector.scalar_tensor_tensor(
                out=d["invnegstep"], in0=nwp[:, 0:1], scalar=-1.0,
                in1=d["bmax"], op0=Alu.mult, op1=Alu.mult,
            )                                                    # -lmax

        # ---------- y0 = project(ones) ----------
        for h in (0, 1):
            d = H[h]
            nc.vector.memset(d["neglam"], -1e30)
            for st in newton_stages(h, d["imu"], d["mu"]):
                st()
            nc.vector.tensor_scalar(
                out=d["lam"], in0=d["neglam"], scalar1=-1.0, scalar2=None,
                op0=Alu.mult,
            )
            nc.vector.tensor_scalar(
                out=d["t"], in0=d["mu"], scalar1=d["lam"][:, 0:1], scalar2=1.0,
                op0=Alu.mult, op1=Alu.add,
            )
            nc.vector.tensor_scalar(
                out=d["ys"], in0=d["t"], scalar1=0.0,
                scalar2=d["negstep"][:, 0:1], op0=Alu.max, op1=Alu.mult,
            )
            transpose_to_xB(h, d["ys"])

        # ---------- PGD (software-pipelined halves) ----------
        # Phase (k, h): matvec_h(k) interleaved with update_{1-h} of its
        # pending iteration.
        pending = {0: None, 1: None}   # half -> iteration index awaiting update
        for k in range(pgd_iters):
            for h in (0, 1):
                other = 1 - h
                up = (pgd_update_stages(other, pending[other])
                      if pending[other] is not None else [])
                run_interleaved(matvec_stages(h, H[h]["g"]), up)
                pending[other] = None
                pending[h] = k
        for h in (0, 1):
            if pending[h] is not None:
                for st in pgd_update_stages(h, pending[h]):
                    st()
                pending[h] = None

        # ---------- postprocess ----------
        for h in (0, 1):
            d = H[h]
            nc.vector.tensor_scalar(
                out=d["prod"][:, 0, :], in0=d["mu"], scalar1=1e-6, scalar2=None,
                op0=Alu.is_gt, op1=Alu.add, accum_out=d["cnt"],
            )
            cntp = nw_pool.tile([64, 2], F32, tag=f"nw{h}", name=f"nw{h}")
            gmm(h, d["cnt"], cntp, 1)
            nc.vector.tensor_scalar(
                out=d["mvd"], in0=cntp[:, 0:1], scalar1=0.5, scalar2=None,
                op0=Alu.is_gt,
            )
            nc.vector.tensor_scalar(
                out=d["omv"], in0=d["mvd"], scalar1=-1.0, scalar2=1.0,
                op0=Alu.mult, op1=Alu.add,
            )
            y2 = d["t"]   # reuse
            nc.vector.tensor_scalar(
                out=y2, in0=d["yfin"], scalar1=d["mvd"][:, 0:1],
                scalar2=d["omv"][:, 0:1], op0=Alu.mult, op1=Alu.add,
            )
            nc.vector.tensor_scalar(
                out=d["prod"][:, 0, :], in0=y2, scalar1=1.0, scalar2=None,
                op0=Alu.mult, op1=Alu.add, accum_out=d["sp"],
            )
            spp = nw_pool.tile([64, 2], F32, tag=f"nw{h}", name=f"nw{h}")
            gmm(h, d["sp"], spp, 1)
            nc.vector.tensor_scalar(
                out=d["ok"], in0=spp[:, 0:1], scalar1=1e-6, scalar2=None,
                op0=Alu.is_gt,
            )
            nc.vector.tensor_scalar(
                out=d["bmax"], in0=spp[:, 0:1], scalar1=1e-30, scalar2=None,
                op0=Alu.max,
            )
            nc.vector.reciprocal(d["rb"], d["bmax"])
            nc.vector.tensor_mul(d["sc"], d["rb"], d["ok"])
            nc.vector.tensor_scalar(
                out=d["off"], in0=d["ok"], scalar1=-1.0 / N, scalar2=1.0 / N,
                op0=Alu.mult, op1=Alu.add,
            )
            w1 = d["u"]   # reuse
            nc.vector.tensor_scalar(
                out=w1, in0=y2, scalar1=d["sc"][:, 0:1],
                scalar2=d["off"][:, 0:1], op0=Alu.mult, op1=Alu.add,
            )
            nc.vector.tensor_scalar(
                out=d["prod"][:, 0, :], in0=w1, scalar1=1.0, scalar2=None,
                op0=Alu.mult, op1=Alu.add, accum_out=d["s2"],
            )
            s2p = nw_pool.tile([64, 2], F32, tag=f"nw{h}", name=f"nw{h}")
            gmm(h, d["s2"], s2p, 1)
            nc.vector.reciprocal(d["rb"], s2p[:, 0:1])
            wf = d["r"]   # reuse
            nc.vector.tensor_scalar(
                out=wf, in0=w1, scalar1=d["rb"][:, 0:1], scalar2=None,
                op0=Alu.mult,
            )
            for Pp, s, dd, b in half_samples(h):
                p0, p1 = a4_sample_range(Pp, s, dd)
                nc.sync.dma_start(out=w_dram[b: b + 1, :],
                                  in_=wf[p0:p1, :])

    nc.compile()
    return nc


def _get_program():
    if "nc" not in _PROGRAM_CACHE:
        _PROGRAM_CACHE["nc"] = _build_program()
    return _PROGRAM_CACHE["nc"]


def _host_inputs(mu, sig):
    import ml_dtypes
    sig_bf = sig.astype(ml_dtypes.bfloat16)
    g64 = np.kron(np.eye(16, dtype=np.float32), np.ones((4, 4), np.float32))
    id64 = np.eye(64, dtype=np.float32)
    return sig_bf, g64, id64


def kernel(predicted_returns: np.ndarray, covariance_matrix: np.ndarray) -> np.ndarray:
    from concourse.bass_utils import run_bass_kernel_spmd

    mu = np.ascontiguousarray(predicted_returns, dtype=np.float32)
    sig = np.ascontiguousarray(covariance_matrix, dtype=np.float32)
    batch = mu.shape[0]
    assert batch == NCORES * SPC and mu.shape[1] == N

    sig_bf, g64, id64 = _host_inputs(mu, sig)

    nc = _get_program()
    in_maps = []
    for c in range(NCORES):
        sl = slice(c * SPC, (c + 1) * SPC)
        in_maps.append({
            "mu_in": mu[sl],
            "sigma_in": sig_bf[sl],
            "g64_in": g64,
            "id64_in": id64,
        })
    res = run_bass_kernel_spmd(nc, in_maps, core_ids=list(range(NCORES)))
    out = np.concatenate([r["w_out"] for r in res.results], axis=0)
    return out.astype(np.float32)


if __name__ == "__main__":
    rng = np.random.default_rng(0)
    mu = (0.05 + 0.1 * rng.random((NCORES * SPC, N))).astype(np.float32)
    A = rng.standard_normal((4, N, N)).astype(np.float32)
    sig = np.einsum("bik,bjk->bij", A, A) / N + 0.1 * np.eye(N, dtype=np.float32)
    sig = np.tile(sig, (64, 1, 1)).astype(np.float32)
    w = kernel(mu, sig)
    print(w.shape, w.sum(axis=1)[:4])
